# revision 1
# baseline (speedup 1.0000x reference)
"""APPNP (MLP + K-hop personalized-propagation + log_softmax) on 8 TRN2 NeuronCores.

Strategy (graph/data parallel, per sharding hint):
- Nodes are permuted (degree-balanced snake deal) and sharded row-wise across
  the 8 cores; W1/W2 are replicated.
- Per hop, each core processes the in-edges of its node shard:
  z rows are fetched from a full replicated copy of z in HBM via
  gpsimd.dma_gather (edge-major, 256B rows), scaled by the gcn norm on DVE,
  and segment-summed per destination with one-hot matmuls on the
  TensorEngine accumulating in PSUM.
- The per-hop "halo exchange" is an AllGather of the 12.5k-row z shard.
- int16 gather indices limit one gather to a 25088-row window of z, so edges
  are grouped by (source-chunk, dest-block); per-(chunk,block) partials are
  accumulated in SBUF.
"""

import sys

sys.path.insert(0, "/opt/trn_rl_repo")

import numpy as np


def kernel(x, edge_index, edge_weight, W1, b1, W2, b2):
    out, _ = appnp_trn(
        np.asarray(x, dtype=np.float32),
        np.asarray(edge_index, dtype=np.int32),
        np.asarray(edge_weight, dtype=np.float32),
        np.asarray(W1, dtype=np.float32),
        np.asarray(b1, dtype=np.float32),
        np.asarray(W2, dtype=np.float32),
        np.asarray(b2, dtype=np.float32),
        k_hops=10,
        alpha=0.1,
    )
    return out


def _host_prep(edge_index, edge_weight, N, ncores, alpha, SHARD, GB, cnt_per_core):
    """Build the permutation, shards and padded slot schedules."""
    NBLK = SHARD // 128
    NPAD = ncores * SHARD
    CH = NPAD // 4  # rows per gather source window (int16 limit)
    assert CH <= 32768 and NPAD % 4 == 0

    row = edge_index[0].astype(np.int64)
    col = edge_index[1].astype(np.int64)
    w = edge_weight.astype(np.float64)
    ar = np.arange(N, dtype=np.int64)
    row_e = np.concatenate([row, ar])
    col_e = np.concatenate([col, ar])
    w_e = np.concatenate([w, np.ones(N)])
    deg = np.bincount(col_e, weights=w_e, minlength=N)
    dis = np.where(deg > 0, 1.0 / np.sqrt(deg), 0.0)
    norm = (dis[row_e] * w_e * dis[col_e] * (1.0 - alpha)).astype(np.float32)

    # snake-deal nodes (sorted by in-edge count) across cores for load balance
    cnt = np.bincount(col_e, minlength=N)
    order = np.argsort(cnt, kind="stable")
    i = np.arange(N)
    r = i // ncores
    j = i % ncores
    core_sorted = np.where(r % 2 == 0, j, ncores - 1 - j)
    core_of = np.empty(N, np.int64)
    pos_of = np.empty(N, np.int64)
    core_of[order] = core_sorted
    pos_of[order] = r
    newid = core_of * SHARD + pos_of
    node_of = np.full((ncores, SHARD), 0, np.int64)
    node_of[core_of, pos_of] = np.arange(N)

    rowp = newid[row_e]
    dest = newid[col_e]
    src_q = rowp // CH
    src_rel = (rowp - src_q * CH).astype(np.int64)

    per_core = []
    counts = np.zeros((ncores, 4 * NBLK), np.int64)
    for c in range(ncores):
        m = (dest >= c * SHARD) & (dest < c * SHARD + cnt_per_core[c])
        dc = dest[m] - c * SHARD
        key = src_q[m] * NBLK + (dc >> 7)
        o = np.argsort(key, kind="stable")
        per_core.append(
            (
                key[o],
                src_rel[m][o].astype(np.int64),
                norm[m][o],
                (dc & 127).astype(np.float32)[o],
            )
        )
        counts[c] = np.bincount(key, minlength=4 * NBLK)

    # common cross-core schedule: groups (of 128 slots) per (chunk, block)
    gqb = -(-counts.max(axis=0) // 128)  # ceil
    gqb = gqb.reshape(4, NBLK)
    gqb[0] = np.maximum(gqb[0], 1)  # q=0 initializes the SBUF accumulator
    # pad each chunk's slot count to a multiple of GB (gather batch)
    for q in range(4):
        Lq = 128 * gqb[q].sum()
        pad = (-Lq) % GB
        gqb[q, NBLK - 1] += pad // 128
    gqb = gqb.reshape(-1)
    group_off = np.concatenate([[0], np.cumsum(128 * gqb)])
    S_tot = int(group_off[-1])
    Lq_arr = [int(128 * gqb.reshape(4, NBLK)[q].sum()) for q in range(4)]
    chunk_off = np.concatenate([[0], np.cumsum(Lq_arr)]).astype(np.int64)

    idx_w = np.zeros((ncores, 128, S_tot // 16), np.int16)
    nrm_t = np.zeros((ncores, 128, S_tot // 128), np.float32)
    lds_t = np.zeros((ncores, 128, S_tot // 128), np.float32)
    for c in range(ncores):
        ks, rels, nrms, ldss = per_core[c]
        first = np.concatenate([[0], np.cumsum(counts[c])[:-1]])
        rank = np.arange(ks.size) - first[ks]
        pos = group_off[ks] + rank
        rel16 = np.zeros(S_tot, np.int16)
        nrm_s = np.zeros(S_tot, np.float32)
        lds_s = np.zeros(S_tot, np.float32)
        rel16[pos] = rels.astype(np.int16)
        nrm_s[pos] = nrms
        lds_s[pos] = ldss
        for q in range(4):
            a, b = int(chunk_off[q]), int(chunk_off[q + 1])
            idx_w[c, :, a // 16 : b // 16] = np.tile(
                rel16[a:b].reshape(-1, 16).T, (8, 1)
            )
            nrm_t[c, :, a // 128 : b // 128] = nrm_s[a:b].reshape(-1, 128).T
            lds_t[c, :, a // 128 : b // 128] = lds_s[a:b].reshape(-1, 128).T

    return dict(
        newid=newid,
        node_of=node_of,
        gqb=gqb.reshape(4, NBLK),
        chunk_off=chunk_off,
        S_tot=S_tot,
        idx_w=idx_w,
        nrm_t=nrm_t,
        lds_t=lds_t,
        CH=CH,
    )


def appnp_trn(
    x,
    edge_index,
    edge_weight,
    W1,
    b1,
    W2,
    b2,
    k_hops,
    alpha,
    GB=1024,
    ncores=8,
    trace=False,
    use_f32r=False,
    use_bf16=True,
    ablate_gather=False,
    ablate_matmul=False,
    ablate_eq=False,
):
    from concourse import bass, bacc, tile, mybir, library_config
    from concourse.bass_utils import run_bass_kernel_spmd

    N, F = x.shape
    H = W1.shape[1]
    C = W2.shape[1]
    assert C == 64 and H <= 128 and GB % 128 == 0

    base = N // ncores
    cnt_per_core = np.full(ncores, base, np.int64)
    cnt_per_core[: N - base * ncores] += 1
    SHARD = -(-int(cnt_per_core.max()) // 128) * 128
    # total padded rows must be divisible by 4 with chunk <= 32768
    while (ncores * SHARD) % 512 != 0:
        SHARD += 128
    NPAD = ncores * SHARD
    NBLK = SHARD // 128
    FP = -(-F // 128) * 128
    KCH = FP // 128

    prep = _host_prep(edge_index, edge_weight, N, ncores, alpha, SHARD, GB, cnt_per_core)
    CH = prep["CH"]
    gqb = prep["gqb"]
    chunk_off = prep["chunk_off"]
    S_tot = prep["S_tot"]
    GPB = GB // 128  # groups per gather batch

    # ---- per-core inputs ----
    in_maps = []
    W1p = np.zeros((FP, H), np.float32)
    W1p[:F] = W1
    b1c = b1.reshape(H, 1).astype(np.float32)
    b2r = np.tile(b2.reshape(1, C), (128, 1)).astype(np.float32)
    iota = np.tile(np.arange(128, dtype=np.float32), (128, 1))
    for c in range(ncores):
        xT = np.zeros((FP, SHARD), np.float32)
        nodes_c = prep["node_of"][c, : cnt_per_core[c]]
        xT[:F, : cnt_per_core[c]] = x[nodes_c].T
        in_maps.append(
            {
                "xT": xT,
                "W1": W1p,
                "b1": b1c,
                "W2": W2.astype(np.float32),
                "b2r": b2r,
                "iota": iota,
                "gidx": prep["idx_w"][c],
                "gnrm": prep["nrm_t"][c],
                "glds": prep["lds_t"][c],
            }
        )

    # ---- build the device program ----
    nc = bacc.Bacc(
        "TRN2", target_bir_lowering=False, debug=False, num_devices=ncores,
        num_swdge_queues=4,
    )
    dt = mybir.dt
    xT_d = nc.dram_tensor("xT", [FP, SHARD], dt.float32, kind="ExternalInput").ap()
    W1_d = nc.dram_tensor("W1", [FP, H], dt.float32, kind="ExternalInput").ap()
    b1_d = nc.dram_tensor("b1", [H, 1], dt.float32, kind="ExternalInput").ap()
    W2_d = nc.dram_tensor("W2", [H, C], dt.float32, kind="ExternalInput").ap()
    b2_d = nc.dram_tensor("b2r", [128, C], dt.float32, kind="ExternalInput").ap()
    io_d = nc.dram_tensor("iota", [128, 128], dt.float32, kind="ExternalInput").ap()
    gi_d = nc.dram_tensor("gidx", [128, S_tot // 16], dt.int16, kind="ExternalInput").ap()
    gn_d = nc.dram_tensor("gnrm", [128, S_tot // 128], dt.float32, kind="ExternalInput").ap()
    gl_d = nc.dram_tensor("glds", [128, S_tot // 128], dt.float32, kind="ExternalInput").ap()
    out_d = nc.dram_tensor("out", [SHARD, C], dt.float32, kind="ExternalOutput").ap()

    rg = [list(range(ncores))]

    with tile.TileContext(nc) as tc:
        nc.gpsimd.load_library(library_config.mlp)
        with (
            tc.tile_pool(name="const", bufs=1) as cst,
            tc.tile_pool(name="resident", bufs=1) as res,
            tc.tile_pool(name="gw", bufs=8) as gw,
            tc.tile_pool(name="rw", bufs=8) as rw,
            tc.tile_pool(name="sm", bufs=4) as sm,
            tc.tile_pool(name="psg", bufs=6, space="PSUM") as psg,
            tc.tile_pool(name="dram", bufs=1, space="DRAM") as dram,
        ):
            # constants / resident data
            W1_t = cst.tile([128, KCH * H], dt.float32)
            for k in range(KCH):
                nc.sync.dma_start(W1_t[:, k * H : (k + 1) * H], W1_d[k * 128 : (k + 1) * 128, :])
            W2_t = cst.tile([H, C], dt.float32)
            nc.sync.dma_start(W2_t[:], W2_d[:])
            b1_t = cst.tile([H, 1], dt.float32)
            nc.sync.dma_start(b1_t[:], b1_d[:])
            b2_t = cst.tile([128, C], dt.float32)
            nc.sync.dma_start(b2_t[:], b2_d[:])
            io_t = cst.tile([128, 128], dt.float32)
            nc.sync.dma_start(io_t[:], io_d[:])
            gi_t = res.tile([128, S_tot // 16], dt.int16)
            nc.sync.dma_start(gi_t[:], gi_d[:])
            gn_t = res.tile([128, S_tot // 128], dt.float32)
            nc.sync.dma_start(gn_t[:], gn_d[:])
            gl_t = res.tile([128, S_tot // 128], dt.float32)
            nc.sync.dma_start(gl_t[:], gl_d[:])
            h01 = res.tile([128, NBLK * C], dt.float32)
            agg = res.tile([128, NBLK * C], dt.float32)

            zsh = dram.tile([SHARD, C], dt.float32)
            zfull = [
                dram.tile([NPAD, C], dt.float32, addr_space="Shared", name=f"zf{k}")
                for k in range(k_hops)
            ]

            # ---- MLP: h = relu(x @ W1 + b1) @ W2 + b2 ----
            with (
                tc.tile_pool(name="mlp", bufs=3) as mlp,
                tc.tile_pool(name="ps1", bufs=1, space="PSUM") as ps1,
                tc.tile_pool(name="ps2", bufs=1, space="PSUM") as ps2,
            ):
              for b in range(NBLK):
                xt = mlp.tile([128, KCH, 128], dt.float32)
                for k in range(KCH):
                    nc.sync.dma_start(
                        xt[:, k, :], xT_d[k * 128 : (k + 1) * 128, b * 128 : (b + 1) * 128]
                    )
                p1 = ps1.tile([H, 128], dt.float32, space="PSUM")
                for k in range(KCH):
                    lw = W1_t[:, k * H : (k + 1) * H]
                    rw_ = xt[:, k, :]
                    if use_f32r:
                        lw = lw.bitcast(dt.float32r)
                        rw_ = rw_.bitcast(dt.float32r)
                    nc.tensor.matmul(
                        p1[:], lhsT=lw, rhs=rw_,
                        start=(k == 0), stop=(k == KCH - 1),
                    )
                rT = mlp.tile([H, 128], dt.float32)
                nc.scalar.activation(rT[:], p1[:], mybir.ActivationFunctionType.Relu, bias=b1_t[:])
                p2 = ps2.tile([128, C], dt.float32, space="PSUM")
                lw2, rw2 = rT[:], W2_t[:]
                if use_f32r:
                    lw2 = lw2.bitcast(dt.float32r)
                    rw2 = rw2.bitcast(dt.float32r)
                nc.tensor.matmul(p2[:], lhsT=lw2, rhs=rw2, start=True, stop=True)
                h = mlp.tile([128, C], dt.float32)
                nc.vector.tensor_tensor(h[:], p2[:], b2_t[:], op=mybir.AluOpType.add)
                nc.scalar.mul(h01[:, b * C : (b + 1) * C], h[:], alpha)
                nc.sync.dma_start(zsh[b * 128 : (b + 1) * 128, :], h[:])

            nc.gpsimd.collective_compute(
                "AllGather", mybir.AluOpType.bypass, replica_groups=rg,
                ins=[zsh[:].opt()], outs=[zfull[0][:].opt()],
            )

            # ---- K propagation hops ----
            for k in range(k_hops):
                zin = zfull[k]
                zout = zfull[k + 1] if k < k_hops - 1 else None
                for q in range(4):
                    a0 = int(chunk_off[q])
                    Lq = int(chunk_off[q + 1]) - a0
                    nbat = Lq // GB
                    zsrc = zin[q * CH : (q + 1) * CH, :]
                    # block schedule for this chunk: group -> block
                    blocks = np.repeat(np.arange(NBLK), gqb[q])
                    pcur = None
                    for bi in range(nbat):
                        s0 = a0 + bi * GB  # global slot offset
                        G = gw.tile([128, GPB, C], dt.float32)
                        if not ablate_gather:
                            nc.gpsimd.dma_gather(
                                out_ap=G[:],
                                in_ap=zsrc,
                                idxs_ap=gi_t[:, s0 // 16 : (s0 + GB) // 16],
                                num_idxs=GB,
                                num_idxs_reg=GB,
                                elem_size=C,
                                queue_num=(s0 // GB) % 4,
                            )
                        mmdt = dt.bfloat16 if use_bf16 else dt.float32
                        G2 = gw.tile([128, GPB, C], mmdt, name="G2", tag="G2")
                        nc.vector.tensor_tensor(
                            G2[:],
                            G[:],
                            gn_t[:, s0 // 128 : (s0 + GB) // 128].unsqueeze(2).to_broadcast([128, GPB, C]),
                            op=mybir.AluOpType.mult,
                        )
                        R = rw.tile([128, GPB, 128], mmdt)
                        if not ablate_eq:
                            nc.vector.tensor_tensor(
                                R[:],
                                io_t[:].unsqueeze(1).to_broadcast([128, GPB, 128]),
                                gl_t[:, s0 // 128 : (s0 + GB) // 128].unsqueeze(2).to_broadcast([128, GPB, 128]),
                                op=mybir.AluOpType.is_equal,
                            )
                        for j in range(GPB):
                            gg = bi * GPB + j  # group index within chunk
                            b = int(blocks[gg])
                            first = gg == 0 or int(blocks[gg - 1]) != b
                            last = gg == len(blocks) - 1 or int(blocks[gg + 1]) != b
                            if first:
                                pcur = psg.tile([128, C], dt.float32, space="PSUM", name="pg", tag="pg")
                            if not ablate_matmul:
                                nc.tensor.matmul(
                                    pcur[:], lhsT=R[:, j, :], rhs=G2[:, j, :],
                                    start=first, stop=last,
                                )
                            elif first:
                                nc.vector.memset(pcur[:], 0.0)
                            if last:
                                if q == 0:
                                    nc.vector.tensor_tensor(
                                        agg[:, b * C : (b + 1) * C], pcur[:],
                                        h01[:, b * C : (b + 1) * C], op=mybir.AluOpType.add,
                                    )
                                else:
                                    nc.vector.tensor_tensor(
                                        agg[:, b * C : (b + 1) * C],
                                        agg[:, b * C : (b + 1) * C],
                                        pcur[:], op=mybir.AluOpType.add,
                                    )
                if k < k_hops - 1:
                    for b in range(NBLK):
                        nc.sync.dma_start(
                            zsh[b * 128 : (b + 1) * 128, :], agg[:, b * C : (b + 1) * C]
                        )
                    nc.gpsimd.collective_compute(
                        "AllGather", mybir.AluOpType.bypass, replica_groups=rg,
                        ins=[zsh[:].opt()], outs=[zout[:].opt()],
                    )

            # ---- log_softmax ----
            for b in range(NBLK):
                zb = agg[:, b * C : (b + 1) * C]
                mx = sm.tile([128, 1], dt.float32)
                nc.vector.tensor_reduce(mx[:], zb, axis=mybir.AxisListType.X, op=mybir.AluOpType.max)
                zc = sm.tile([128, C], dt.float32)
                nc.vector.tensor_scalar(zc[:], zb, mx[:], None, op0=mybir.AluOpType.subtract)
                e = sm.tile([128, C], dt.float32)
                nc.scalar.activation(e[:], zc[:], mybir.ActivationFunctionType.Exp)
                s = sm.tile([128, 1], dt.float32)
                nc.vector.tensor_reduce(s[:], e[:], axis=mybir.AxisListType.X, op=mybir.AluOpType.add)
                ls = sm.tile([128, 1], dt.float32)
                nc.scalar.activation(ls[:], s[:], mybir.ActivationFunctionType.Ln)
                o = sm.tile([128, C], dt.float32)
                nc.vector.tensor_scalar(o[:], zc[:], ls[:], None, op0=mybir.AluOpType.subtract)
                nc.sync.dma_start(out_d[b * 128 : (b + 1) * 128, :], o[:])

    nc.compile()
    res_ = run_bass_kernel_spmd(nc, in_maps, core_ids=list(range(ncores)), trace=trace)

    out = np.empty((N, C), np.float32)
    for c in range(ncores):
        nodes_c = prep["node_of"][c, : cnt_per_core[c]]
        out[nodes_c] = res_.results[c]["out"][: cnt_per_core[c]]
    return out, res_



# revision 12
# speedup vs baseline: 1.5088x; 1.5088x over previous
"""APPNP (MLP + K-hop personalized-propagation + log_softmax) on 8 TRN2 NeuronCores.

Strategy (graph/data parallel, per sharding hint):
- Nodes are sharded row-wise across the 8 cores with an iterative greedy
  assignment that simultaneously balances (a) in-edges per destination shard
  and (b) in-edge counts per (source-window, dest-block) pair across cores —
  the latter minimizes the padding of the common SPMD gather schedule.
- W1/W2 are replicated.
- Per hop, each core processes the in-edges of its node shard:
  z rows are fetched from a full replicated copy of z in HBM via
  gpsimd.dma_gather (edge-major, 256B rows), scaled by the gcn norm on DVE,
  and segment-summed per destination with one-hot matmuls on the
  TensorEngine accumulating in PSUM.
- The per-hop "halo exchange" is an AllGather of the 12.5k-row z shard.
- int16 gather indices limit one gather to a 25088-row window of z, so edges
  are grouped by (source-window, dest-block); per-(window,block) partials are
  accumulated in SBUF.
"""

import sys

sys.path.insert(0, "/opt/trn_rl_repo")

import numpy as np


def kernel(x, edge_index, edge_weight, W1, b1, W2, b2):
    out, _ = appnp_trn(
        np.asarray(x, dtype=np.float32),
        np.asarray(edge_index, dtype=np.int32),
        np.asarray(edge_weight, dtype=np.float32),
        np.asarray(W1, dtype=np.float32),
        np.asarray(b1, dtype=np.float32),
        np.asarray(W2, dtype=np.float32),
        np.asarray(b2, dtype=np.float32),
        k_hops=10,
        alpha=0.1,
    )
    return out


def _assign_cores(row_e, col_e, N, ncores, SHARD, rounds=3):
    """Assign nodes to (core, position) balancing per-(window, block) in-edge
    counts across cores. Window of a source node = its core // 2 (CH = 2*SHARD),
    which depends on this very assignment -> iterate."""
    cnt = np.bincount(col_e, minlength=N)
    order = np.argsort(cnt, kind="stable")  # ascending in-degree
    nstrata = N // ncores
    strata = order.reshape(nstrata, ncores)  # similar-degree groups of 8
    pos_of = np.empty(N, np.int64)
    pos_of[order] = np.repeat(np.arange(nstrata), ncores)

    # initial snake assignment
    core_of = np.empty(N, np.int64)
    i = np.arange(N)
    r = i // ncores
    j = i % ncores
    core_of[order] = np.where((r % 2 == 0)[:, None] if False else (r % 2 == 0), j, ncores - 1 - j)

    nq = 4
    for _ in range(rounds):
        # per-node in-degree split by source window (= src core // 2)
        src_q = core_of[row_e] // 2
        dq = np.zeros((N, nq), np.float64)
        np.add.at(dq, (col_e, src_q), 1.0)
        new_core = np.empty(N, np.int64)
        dqs = dq[strata]  # [nstrata, 8, 4]
        tot = dqs.sum(axis=2)  # [nstrata, 8]
        ordd = np.argsort(-tot, axis=1)
        for b0 in range(0, nstrata, 128):
            running = np.zeros((ncores, nq))
            for rr in range(b0, min(b0 + 128, nstrata)):
                used = 0
                nodes = strata[rr]
                for k in ordd[rr]:
                    n = nodes[k]
                    best, bestv = -1, None
                    cand = running + dq[n]
                    mx = cand.max(axis=1) + 1e-3 * cand.sum(axis=1)
                    for c in range(ncores):
                        if used >> c & 1:
                            continue
                        if best < 0 or mx[c] < bestv:
                            best, bestv = c, mx[c]
                    new_core[n] = best
                    used |= 1 << best
                    running[best] += dq[n]
        core_of = new_core
    return core_of, pos_of


def _host_prep(edge_index, edge_weight, N, ncores, alpha, SHARD, GB, cnt_per_core):
    """Build the permutation, shards and padded slot schedules."""
    NBLK = SHARD // 128
    NPAD = ncores * SHARD
    CH = NPAD // 4  # rows per gather source window (int16 limit)
    assert CH <= 32768 and NPAD % 4 == 0

    # self-loops are NOT gathered: their contribution nsl[i]*z[i] is computed
    # on-chip from the resident agg tile (cuts schedule padding: a node's
    # self-loop otherwise always lands in its own core's source window).
    row_e = edge_index[0].astype(np.int64)
    col_e = edge_index[1].astype(np.int64)
    w_e = edge_weight.astype(np.float64)
    ar = np.arange(N, dtype=np.int64)
    deg = np.bincount(np.concatenate([col_e, ar]),
                      weights=np.concatenate([w_e, np.ones(N)]), minlength=N)
    dis = np.where(deg > 0, 1.0 / np.sqrt(deg), 0.0)
    norm = (dis[row_e] * w_e * dis[col_e] * (1.0 - alpha)).astype(np.float32)
    nsl = ((1.0 - alpha) * dis * dis).astype(np.float32)  # self-loop weight

    core_of, pos_of = _assign_cores(row_e, col_e, N, ncores, SHARD, rounds=4)
    newid = core_of * SHARD + pos_of
    node_of = np.full((ncores, SHARD), 0, np.int64)
    node_of[core_of, pos_of] = np.arange(N)

    rowp = newid[row_e]
    dest = newid[col_e]
    src_q = rowp // CH
    src_rel = (rowp - src_q * CH).astype(np.int64)

    per_core = []
    counts = np.zeros((ncores, 4 * NBLK), np.int64)
    for c in range(ncores):
        m = (dest >= c * SHARD) & (dest < c * SHARD + cnt_per_core[c])
        dc = dest[m] - c * SHARD
        key = src_q[m] * NBLK + (dc >> 7)
        o = np.argsort(key, kind="stable")
        per_core.append(
            (
                key[o],
                src_rel[m][o].astype(np.int64),
                norm[m][o],
                (dc & 127).astype(np.float32)[o],
            )
        )
        counts[c] = np.bincount(key, minlength=4 * NBLK)

    # common cross-core schedule: groups (of 128 slots) per (chunk, block)
    gqb = -(-counts.max(axis=0) // 128)  # ceil
    gqb = gqb.reshape(4, NBLK)
    gqb[0] = np.maximum(gqb[0], 1)  # q=0 initializes the SBUF accumulator
    # pad each chunk's slot count to a multiple of GB (gather batch)
    for q in range(4):
        Lq = 128 * gqb[q].sum()
        pad = (-Lq) % GB
        gqb[q, NBLK - 1] += pad // 128
    gqb = gqb.reshape(-1)
    group_off = np.concatenate([[0], np.cumsum(128 * gqb)])
    S_tot = int(group_off[-1])
    Lq_arr = [int(128 * gqb.reshape(4, NBLK)[q].sum()) for q in range(4)]
    chunk_off = np.concatenate([[0], np.cumsum(Lq_arr)]).astype(np.int64)

    idx_w = np.zeros((ncores, 128, S_tot // 16), np.int16)
    nrm_t = np.zeros((ncores, 128, S_tot // 128), np.float32)
    lds_t = np.zeros((ncores, 128, S_tot // 128), np.float32)
    for c in range(ncores):
        ks, rels, nrms, ldss = per_core[c]
        first = np.concatenate([[0], np.cumsum(counts[c])[:-1]])
        rank = np.arange(ks.size) - first[ks]
        pos = group_off[ks] + rank
        rel16 = np.zeros(S_tot, np.int16)
        nrm_s = np.zeros(S_tot, np.float32)
        lds_s = np.zeros(S_tot, np.float32)
        rel16[pos] = rels.astype(np.int16)
        nrm_s[pos] = nrms
        lds_s[pos] = ldss
        for q in range(4):
            a, b = int(chunk_off[q]), int(chunk_off[q + 1])
            idx_w[c, :, a // 16 : b // 16] = np.tile(
                rel16[a:b].reshape(-1, 16).T, (8, 1)
            )
            nrm_t[c, :, a // 128 : b // 128] = nrm_s[a:b].reshape(-1, 128).T
            lds_t[c, :, a // 128 : b // 128] = lds_s[a:b].reshape(-1, 128).T

    nsl_t = np.zeros((ncores, 128, NBLK), np.float32)
    for c in range(ncores):
        nodes_c = node_of[c, : cnt_per_core[c]]
        pos = np.arange(cnt_per_core[c])
        nsl_t[c, pos & 127, pos >> 7] = nsl[nodes_c]

    return dict(
        newid=newid,
        node_of=node_of,
        gqb=gqb.reshape(4, NBLK),
        chunk_off=chunk_off,
        S_tot=S_tot,
        idx_w=idx_w,
        nrm_t=nrm_t,
        lds_t=lds_t,
        nsl_t=nsl_t,
        CH=CH,
    )


def appnp_trn(
    x,
    edge_index,
    edge_weight,
    W1,
    b1,
    W2,
    b2,
    k_hops,
    alpha,
    GB=1024,
    ncores=8,
    trace=False,
    use_f32r=False,
    use_bf16=True,
    nqueues=4,
):
    from concourse import bass, bacc, tile, mybir, library_config
    from concourse.bass_utils import run_bass_kernel_spmd

    N, F = x.shape
    H = W1.shape[1]
    C = W2.shape[1]
    assert C == 64 and H <= 128 and GB % 128 == 0

    base = N // ncores
    cnt_per_core = np.full(ncores, base, np.int64)
    cnt_per_core[: N - base * ncores] += 1
    SHARD = -(-int(cnt_per_core.max()) // 128) * 128
    # total padded rows must be divisible by 4 with chunk <= 32768
    while (ncores * SHARD) % 512 != 0:
        SHARD += 128
    NPAD = ncores * SHARD
    NBLK = SHARD // 128
    FP = -(-F // 128) * 128
    KCH = FP // 128

    prep = _host_prep(edge_index, edge_weight, N, ncores, alpha, SHARD, GB, cnt_per_core)
    CH = prep["CH"]
    gqb = prep["gqb"]
    chunk_off = prep["chunk_off"]
    S_tot = prep["S_tot"]
    GPB = GB // 128  # groups per gather batch

    # ---- per-core inputs ----
    in_maps = []
    W1p = np.zeros((FP, H), np.float32)
    W1p[:F] = W1
    b1c = b1.reshape(H, 1).astype(np.float32)
    b2r = np.tile(b2.reshape(1, C), (128, 1)).astype(np.float32)
    iota = np.tile(np.arange(128, dtype=np.float32), (128, 1))
    for c in range(ncores):
        xT = np.zeros((FP, SHARD), np.float32)
        nodes_c = prep["node_of"][c, : cnt_per_core[c]]
        xT[:F, : cnt_per_core[c]] = x[nodes_c].T
        in_maps.append(
            {
                "xT": xT,
                "W1": W1p,
                "b1": b1c,
                "W2": W2.astype(np.float32),
                "b2r": b2r,
                "iota": iota,
                "nsl": prep["nsl_t"][c],
                "gidx": prep["idx_w"][c],
                "gnrm": prep["nrm_t"][c],
                "glds": prep["lds_t"][c],
            }
        )

    # ---- build the device program ----
    nc = bacc.Bacc(
        "TRN2", target_bir_lowering=False, debug=False, num_devices=ncores,
        num_swdge_queues=nqueues,
    )
    dt = mybir.dt
    xT_d = nc.dram_tensor("xT", [FP, SHARD], dt.float32, kind="ExternalInput").ap()
    W1_d = nc.dram_tensor("W1", [FP, H], dt.float32, kind="ExternalInput").ap()
    b1_d = nc.dram_tensor("b1", [H, 1], dt.float32, kind="ExternalInput").ap()
    W2_d = nc.dram_tensor("W2", [H, C], dt.float32, kind="ExternalInput").ap()
    b2_d = nc.dram_tensor("b2r", [128, C], dt.float32, kind="ExternalInput").ap()
    io_d = nc.dram_tensor("iota", [128, 128], dt.float32, kind="ExternalInput").ap()
    ns_d = nc.dram_tensor("nsl", [128, SHARD // 128], dt.float32, kind="ExternalInput").ap()
    gi_d = nc.dram_tensor("gidx", [128, S_tot // 16], dt.int16, kind="ExternalInput").ap()
    gn_d = nc.dram_tensor("gnrm", [128, S_tot // 128], dt.float32, kind="ExternalInput").ap()
    gl_d = nc.dram_tensor("glds", [128, S_tot // 128], dt.float32, kind="ExternalInput").ap()
    out_d = nc.dram_tensor("out", [SHARD, C], dt.float32, kind="ExternalOutput").ap()

    rg = [list(range(ncores))]

    with tile.TileContext(nc) as tc:
        nc.gpsimd.load_library(library_config.mlp)
        with (
            tc.tile_pool(name="const", bufs=1) as cst,
            tc.tile_pool(name="resident", bufs=1) as res,
            tc.tile_pool(name="gw", bufs=7) as gw,
            tc.tile_pool(name="rw", bufs=6) as rw,
            tc.tile_pool(name="sm", bufs=4) as sm,
            tc.tile_pool(name="psg", bufs=6, space="PSUM") as psg,
            tc.tile_pool(name="dram", bufs=1, space="DRAM") as dram,
        ):
            # constants / resident data
            W1_t = cst.tile([128, KCH * H], dt.float32)
            for k in range(KCH):
                nc.sync.dma_start(W1_t[:, k * H : (k + 1) * H], W1_d[k * 128 : (k + 1) * 128, :])
            W2_t = cst.tile([H, C], dt.float32)
            nc.sync.dma_start(W2_t[:], W2_d[:])
            b1_t = cst.tile([H, 1], dt.float32)
            nc.sync.dma_start(b1_t[:], b1_d[:])
            b2_t = cst.tile([128, C], dt.float32)
            nc.sync.dma_start(b2_t[:], b2_d[:])
            io_t = cst.tile([128, 128], dt.float32)
            nc.sync.dma_start(io_t[:], io_d[:])
            ns_t = cst.tile([128, NBLK], dt.float32)
            nc.sync.dma_start(ns_t[:], ns_d[:])
            gi_t = res.tile([128, S_tot // 16], dt.int16)
            nc.sync.dma_start(gi_t[:], gi_d[:])
            gn_t = res.tile([128, S_tot // 128], dt.float32)
            nc.sync.dma_start(gn_t[:], gn_d[:])
            gl_t = res.tile([128, S_tot // 128], dt.float32)
            nc.sync.dma_start(gl_t[:], gl_d[:])
            h01 = res.tile([128, NBLK, C], dt.float32)
            agg = res.tile([128, NBLK, C], dt.float32)
            slf = res.tile([128, NBLK, C], dt.float32)

            zsh = dram.tile([SHARD, C], dt.float32)
            zfull = [
                dram.tile([NPAD, C], dt.float32, addr_space="Shared", name=f"zf{k}")
                for k in range(k_hops)
            ]

            # ---- MLP: h = relu(x @ W1 + b1) @ W2 + b2 ----
            with (
                tc.tile_pool(name="mlp", bufs=3) as mlp,
                tc.tile_pool(name="ps1", bufs=1, space="PSUM") as ps1,
                tc.tile_pool(name="ps2", bufs=1, space="PSUM") as ps2,
            ):
              for b in range(NBLK):
                xt = mlp.tile([128, KCH, 128], dt.float32)
                for k in range(KCH):
                    nc.sync.dma_start(
                        xt[:, k, :], xT_d[k * 128 : (k + 1) * 128, b * 128 : (b + 1) * 128]
                    )
                p1 = ps1.tile([H, 128], dt.float32, space="PSUM")
                for k in range(KCH):
                    lw = W1_t[:, k * H : (k + 1) * H]
                    rw_ = xt[:, k, :]
                    if use_f32r:
                        lw = lw.bitcast(dt.float32r)
                        rw_ = rw_.bitcast(dt.float32r)
                    nc.tensor.matmul(
                        p1[:], lhsT=lw, rhs=rw_,
                        start=(k == 0), stop=(k == KCH - 1),
                    )
                rT = mlp.tile([H, 128], dt.float32)
                nc.scalar.activation(rT[:], p1[:], mybir.ActivationFunctionType.Relu, bias=b1_t[:])
                p2 = ps2.tile([128, C], dt.float32, space="PSUM")
                lw2, rw2 = rT[:], W2_t[:]
                if use_f32r:
                    lw2 = lw2.bitcast(dt.float32r)
                    rw2 = rw2.bitcast(dt.float32r)
                nc.tensor.matmul(p2[:], lhsT=lw2, rhs=rw2, start=True, stop=True)
                h = mlp.tile([128, C], dt.float32)
                nc.vector.tensor_tensor(h[:], p2[:], b2_t[:], op=mybir.AluOpType.add)
                nc.scalar.mul(h01[:, b, :], h[:], alpha)
                nc.vector.tensor_scalar(slf[:, b, :], h[:], ns_t[:, b : b + 1], None, op0=mybir.AluOpType.mult)
                nc.vector.tensor_tensor(slf[:, b, :], slf[:, b, :], h01[:, b, :], op=mybir.AluOpType.add)
                nc.sync.dma_start(zsh[b * 128 : (b + 1) * 128, :], h[:])

            nc.gpsimd.collective_compute(
                "AllGather", mybir.AluOpType.bypass, replica_groups=rg,
                ins=[zsh[:].opt()], outs=[zfull[0][:].opt()],
            )

            # ---- K propagation hops ----
            for k in range(k_hops):
                zin = zfull[k]
                zout = zfull[k + 1] if k < k_hops - 1 else None
                for q in range(4):
                    a0 = int(chunk_off[q])
                    Lq = int(chunk_off[q + 1]) - a0
                    nbat = Lq // GB
                    zsrc = zin[q * CH : (q + 1) * CH, :]
                    # block schedule for this chunk: group -> block
                    blocks = np.repeat(np.arange(NBLK), gqb[q])
                    pcur = None
                    for bi in range(nbat):
                        s0 = a0 + bi * GB  # global slot offset
                        G = gw.tile([128, GPB, C], dt.float32)
                        nc.gpsimd.dma_gather(
                            out_ap=G[:],
                            in_ap=zsrc,
                            idxs_ap=gi_t[:, s0 // 16 : (s0 + GB) // 16],
                            num_idxs=GB,
                            num_idxs_reg=GB,
                            elem_size=C,
                            queue_num=(s0 // GB) % nqueues,
                        )
                        mmdt = dt.bfloat16 if use_bf16 else dt.float32
                        G2 = gw.tile([128, GPB, C], mmdt, name="G2", tag="G2")
                        nc.vector.tensor_tensor(
                            G2[:],
                            G[:],
                            gn_t[:, s0 // 128 : (s0 + GB) // 128].unsqueeze(2).to_broadcast([128, GPB, C]),
                            op=mybir.AluOpType.mult,
                        )
                        R = rw.tile([128, GPB, 128], mmdt)
                        nc.vector.tensor_tensor(
                            R[:],
                            io_t[:].unsqueeze(1).to_broadcast([128, GPB, 128]),
                            gl_t[:, s0 // 128 : (s0 + GB) // 128].unsqueeze(2).to_broadcast([128, GPB, 128]),
                            op=mybir.AluOpType.is_equal,
                        )
                        for j in range(GPB):
                            gg = bi * GPB + j  # group index within chunk
                            b = int(blocks[gg])
                            first = gg == 0 or int(blocks[gg - 1]) != b
                            last = gg == len(blocks) - 1 or int(blocks[gg + 1]) != b
                            if first:
                                pcur = psg.tile([128, C], dt.float32, space="PSUM", name="pg", tag="pg")
                            nc.tensor.matmul(
                                pcur[:], lhsT=R[:, j, :], rhs=G2[:, j, :],
                                start=first, stop=last,
                            )
                            if last:
                                if q == 0:
                                    nc.vector.tensor_tensor(
                                        agg[:, b, :], pcur[:],
                                        slf[:, b, :], op=mybir.AluOpType.add,
                                    )
                                else:
                                    nc.vector.tensor_tensor(
                                        agg[:, b, :],
                                        agg[:, b, :],
                                        pcur[:], op=mybir.AluOpType.add,
                                    )
                if k < k_hops - 1:
                    nc.vector.tensor_tensor(
                        slf[:], agg[:],
                        ns_t[:].unsqueeze(2).to_broadcast([128, NBLK, C]),
                        op=mybir.AluOpType.mult,
                    )
                    nc.vector.tensor_tensor(slf[:], slf[:], h01[:], op=mybir.AluOpType.add)
                    for b in range(NBLK):
                        nc.sync.dma_start(
                            zsh[b * 128 : (b + 1) * 128, :], agg[:, b, :]
                        )
                    nc.gpsimd.collective_compute(
                        "AllGather", mybir.AluOpType.bypass, replica_groups=rg,
                        ins=[zsh[:].opt()], outs=[zout[:].opt()],
                    )

            # ---- log_softmax ----
            for b in range(NBLK):
                zb = agg[:, b, :]
                mx = sm.tile([128, 1], dt.float32)
                nc.vector.tensor_reduce(mx[:], zb, axis=mybir.AxisListType.X, op=mybir.AluOpType.max)
                zc = sm.tile([128, C], dt.float32)
                nc.vector.tensor_scalar(zc[:], zb, mx[:], None, op0=mybir.AluOpType.subtract)
                e = sm.tile([128, C], dt.float32)
                nc.scalar.activation(e[:], zc[:], mybir.ActivationFunctionType.Exp)
                s = sm.tile([128, 1], dt.float32)
                nc.vector.tensor_reduce(s[:], e[:], axis=mybir.AxisListType.X, op=mybir.AluOpType.add)
                ls = sm.tile([128, 1], dt.float32)
                nc.scalar.activation(ls[:], s[:], mybir.ActivationFunctionType.Ln)
                o = sm.tile([128, C], dt.float32)
                nc.vector.tensor_scalar(o[:], zc[:], ls[:], None, op0=mybir.AluOpType.subtract)
                nc.sync.dma_start(out_d[b * 128 : (b + 1) * 128, :], o[:])

    nc.compile()
    res_ = run_bass_kernel_spmd(nc, in_maps, core_ids=list(range(ncores)), trace=trace)

    out = np.empty((N, C), np.float32)
    for c in range(ncores):
        nodes_c = prep["node_of"][c, : cnt_per_core[c]]
        out[nodes_c] = res_.results[c]["out"][: cnt_per_core[c]]
    return out, res_
